# revision 51
# baseline (speedup 1.0000x reference)
"""ConvVMamba TRN2 Bass kernel.

Sharding: data-parallel over batch. B=8 -> one image per NeuronCore, all
weights replicated, no collectives.

Per-core layout: channels on SBUF partitions (C=96), pixels on the free dim
(L=64*64=4096).
  - 1x1 convs / projections: PE matmuls (lhsT = W^T, rhs = activations).
  - depthwise 7x7 / 3x3: PE accumulating matmuls with per-tap diagonal lhsT
    over a zero-padded image buffer.
  - LayerNorm over channels: partition reduction via ones-vector matmul into
    [96,512] stats, Rsqrt ACT for the inverse std, ones-matmul broadcast.
  - selective scan (d_state=1): DVE tensor_tensor_scan (state = a*state + b),
    reversed directions via negative-step APs, transposed directions by
    keeping the whole per-direction pipeline in w-major order.
  - dt/B/C projections fused host-side into per-direction [96,96] lhsT
    acting directly on v4 (no xdbl intermediate).
  - softplus via sigmoid+ln: s = sigmoid(-z), delta' = ln(s) = -softplus(z);
    the sign is folded into the B weights; when A == -1 (d_state=1 init)
    dA = exp(A*softplus(z)) = s, so the scan coefficient is free.
  - ACT function-table discipline: only {rsqrt, gelu, silu, sigmoid, ln,
    exp} need table sets; ops are emitted (and fenced) in set-contiguous
    phases so only ~9 table loads happen instead of one per op.
Branch tensors in bf16, residual stream in fp32.
"""

import sys
import numpy as np

sys.path.insert(0, "/opt/trn_rl_repo")

import ml_dtypes  # noqa: E402
import concourse.bass as bass  # noqa: E402
import concourse.bacc as bacc  # noqa: E402
import concourse.mybir as mybir  # noqa: E402
from concourse import tile  # noqa: E402
from concourse.tile import add_dep_helper  # noqa: E402
from concourse.bass_utils import run_bass_kernel_spmd  # noqa: E402

F32 = mybir.dt.float32
F32R = mybir.dt.float32r
BF16 = mybir.dt.bfloat16
FP8 = mybir.dt.float8e4
PM = mybir.MatmulPerfMode
AF = mybir.ActivationFunctionType
OP = mybir.AluOpType
bfnp = ml_dtypes.bfloat16
f8np = ml_dtypes.float8_e4m3fn
CSCALE = 2048.0  # conv7 fp8 weight scale (power of 2; dequant in bias ACT)

B, C, H, W = 8, 96, 64, 64
L = H * W
R, N, K = 6, 1, 4
EPS = 1e-5
P7, P3 = 70, 66  # padded widths for 7x7 and 3x3 convs
NCHUNK = 8  # 4096 / 512
CH = 512

_CACHE = {}


def _taps(k):
    r = (k - 1) // 2
    return [(dh, dw) for dh in range(k) for dw in range(k)], r


def build_host_tensors(kw):
    """Precompute all weight/constant DRAM tensors (shared across cores)."""
    f = lambda a: np.asarray(a, np.float32)
    out = {}

    # --- fold LN gamma/beta into following 1x1 convs ---
    def fold(wname, bname, g, b):
        w = f(kw[wname])
        bb = f(kw[bname])
        return w * f(g)[None, :], bb + w @ f(b)

    fc1w, fc1b = fold("cn_fc1_w", "cn_fc1_b", kw["cn_ln_w"], kw["cn_ln_b"])
    ipw, ipb = fold("ip_w", "ip_b", kw["v_ln1_w"], kw["v_ln1_b"])
    opw, opb = fold("op_w", "op_b", kw["o_ln_w"], kw["o_ln_b"])
    mfc1w, mfc1b = fold("m_fc1_w", "m_fc1_b", kw["v_ln2_w"], kw["v_ln2_b"])
    fc2w, fc2b = f(kw["cn_fc2_w"]), f(kw["cn_fc2_b"])
    mfc2w, mfc2b = f(kw["m_fc2_w"]), f(kw["m_fc2_b"])

    # --- depthwise 7x7: fp8 DoubleRow pair weights [96, 28*(2*96)] ---
    # pair pr = dh*4 + jw covers taps (dh, 2jw) and (dh, 2jw+1); dw=7 is a
    # phantom zero tap. Weights scaled by CSCALE for fp8 range; the conv
    # output ACT applies 1/CSCALE.
    w7 = f(kw["cn_dw_w"])  # [C, 7, 7]
    ar = np.arange(C)
    wdr7 = np.zeros((C, 28, 2, C), np.float32)
    for dh in range(7):
        for jw in range(4):
            pr = dh * 4 + jw
            wdr7[ar, pr, 0, ar] = w7[:, dh, 2 * jw] * CSCALE
            if 2 * jw + 1 < 7:
                wdr7[ar, pr, 1, ar] = w7[:, dh, 2 * jw + 1] * CSCALE
    out["wdr7"] = wdr7.reshape(C, 28 * 2 * C).astype(f8np)

    # --- depthwise 3x3 diagonals: [96, 9*96] bf16 ---
    w3 = f(kw["dw_w"]).reshape(C, 9)
    diag = np.zeros((C, 9 * C), np.float32)
    for t in range(9):
        diag[ar, t * C + ar] = w3[:, t]
    out["wdiag"] = diag.astype(bfnp)

    # --- GEMM weights (lhsT layouts), bf16 ---
    out["wfc1"] = fc1w.T.astype(bfnp)  # [96, 384]
    wfc2 = np.zeros((128, 3 * C), np.float32)  # [128, 288] K-chunks
    for j in range(3):
        wfc2[:, j * C:(j + 1) * C] = fc2w[:, j * 128:(j + 1) * 128].T
    out["wfc2"] = wfc2.astype(bfnp)
    out["wip"] = ipw.T.astype(bfnp)  # [96, 96]
    out["wop"] = opw.T.astype(bfnp)
    out["wmfc1"] = mfc1w.T.astype(bfnp)
    wm2 = np.zeros((128, 3 * C), np.float32)
    for j in range(3):
        wm2[:, j * C:(j + 1) * C] = mfc2w[:, j * 128:(j + 1) * 128].T
    out["wmfc2"] = wm2.astype(bfnp)

    # --- fused per-direction dt projection and B/C broadcast lhsT ---
    # delta_raw_k = (dt_w[k] @ x_proj_w[k][:R]) @ v4  -> lhsT [96, 96]
    # B_bcast_k[m,p] = x_proj_w[k][R] . v4[:,p]  (negated: softplus sign fold)
    # C_bcast_k[m,p] = x_proj_w[k][R+N] . v4[:,p]
    xp = f(kw["x_proj_w"])  # [4, 8, 96]
    dtw = f(kw["dt_w"])  # [4, 96, 6]
    wdt2 = np.zeros((C, 4 * C), np.float32)
    wbc = np.zeros((C, 8 * C), np.float32)
    for k in range(4):
        wdt2[:, k * C:(k + 1) * C] = (dtw[k] @ xp[k][:R]).T
        wbc[:, (2 * k) * C:(2 * k + 1) * C] = np.repeat(
            -xp[k][R][:, None], C, axis=1)
        wbc[:, (2 * k + 1) * C:(2 * k + 2) * C] = np.repeat(
            xp[k][R + N][:, None], C, axis=1)
    out["wdt2"] = wdt2.astype(bfnp)
    out["wbc"] = wbc.astype(bfnp)

    # ones for LN partition reduce+broadcast
    out["ones96"] = np.ones((C, C), np.float32)
    out["ones96_bf"] = np.ones((C, C), bfnp)

    # per-partition scalar bank [128, NV] fp32
    A = (-np.exp(f(kw["A_logs"]))).reshape(K, C)
    Ds = f(kw["Ds"]).reshape(K, C)
    dtb = f(kw["dt_b"])  # [4, 96]
    cols = []

    def col(v, n=C):
        a = np.zeros(128, np.float32)
        a[: len(v)] = v
        cols.append(a)
        return len(cols) - 1

    ix = {}
    ix["cn_dw_b"] = col(f(kw["cn_dw_b"]))
    for j in range(3):
        ix[f"fc1b{j}"] = col(fc1b[j * 128:(j + 1) * 128])
    ix["fc2b"] = col(fc2b)
    ix["ipb"] = col(ipb)
    ix["dwb"] = col(f(kw["dw_b"]))
    for k in range(4):
        ix[f"dtbn{k}"] = col(-dtb[k])   # sigmoid(-(z+dtb)) bias
        ix[f"An{k}"] = col(-A[k])       # general path: dA = exp(-A * delta')
    ix["Dsum"] = col(Ds.sum(0))
    ix["eps"] = col(np.full(128, EPS, np.float32), 128)
    ix["opb"] = col(opb)
    for j in range(3):
        ix[f"mfc1b{j}"] = col(mfc1b[j * 128:(j + 1) * 128])
    ix["mfc2b"] = col(mfc2b)
    out["vecs"] = np.stack(cols, axis=1)  # [128, NV]
    a_is_neg1 = bool(np.allclose(A, -1.0, atol=0.0, rtol=0.0))
    return out, ix, a_is_neg1


def pad_image(x):
    """[96,64,64] fp32 -> dup-interleaved fp8 [96, 2*70*70].

    xint[c, 2q] = xpad[c, q]; xint[c, 2q+1] = xpad[c, q+1] so any
    horizontally-adjacent tap pair is a 2-byte-aligned fp8 pair for
    DoubleRow matmuls.
    """
    xp = np.zeros((C, P7, P7), np.float32)
    xp[:, 3:3 + H, 3:3 + W] = x
    flat = np.zeros((C, P7 * P7 + 1), np.float32)
    flat[:, :P7 * P7] = xp.reshape(C, P7 * P7)
    xint = np.empty((C, 2 * P7 * P7), np.float32)
    xint[:, 0::2] = flat[:, :-1]
    xint[:, 1::2] = flat[:, 1:]
    return xint.astype(f8np)


def r32(ap):
    return ap.bitcast(F32R)


def build_program(ix, a_is_neg1):
    nc = bacc.Bacc("TRN2", target_bir_lowering=False, debug=False)

    din = {}
    for name, shape, dt in [
        ("xint", [C, 2 * P7 * P7], FP8),
        ("xres", [C, L], F32R),
        ("wdr7", [C, 28 * 2 * C], FP8),
        ("wdiag", [C, 9 * C], BF16),
        ("wfc1", [C, 384], BF16),
        ("wfc2", [128, 3 * C], BF16),
        ("wip", [C, C], BF16),
        ("wop", [C, C], BF16),
        ("wmfc1", [C, 384], BF16),
        ("wmfc2", [128, 3 * C], BF16),
        ("wdt2", [C, 4 * C], BF16),
        ("wbc", [C, 8 * C], BF16),
        ("ones96", [C, C], F32R),
        ("ones96_bf", [C, C], BF16),
        ("vecs", [128, len(ix)], F32),
    ]:
        din[name] = nc.dram_tensor(name, shape, dt, kind="ExternalInput").ap()
    dout = nc.dram_tensor("out", [C, L], F32, kind="ExternalOutput").ap()

    class ActPhase:
        # Fences set-specific ACT ops so the scheduler cannot interleave
        # ops from different activation-table sets (each flip costs an ACT
        # function-table reload). Phase-barrier semantics: each op depends
        # on every op of the PREVIOUS phase, but ops within one phase are
        # unordered so the scheduler can run them by data readiness.
        def __init__(self):
            self.prev_phase = []
            self.cur_phase = []
            self.cur_set = None

        def tag(self, bi, fset):
            inst = bi.ins
            if fset != self.cur_set:
                self.prev_phase = self.cur_phase
                self.cur_phase = []
                self.cur_set = fset
            for p in self.prev_phase:
                add_dep_helper(inst, p, sync=True,
                               reason="act table-set phase fence")
            self.cur_phase.append(inst)
            return bi

    ph = ActPhase()

    with tile.TileContext(nc) as tc:
        from contextlib import ExitStack

        with ExitStack() as ctx:
            const = ctx.enter_context(tc.tile_pool(name="const", bufs=1))
            bigp = ctx.enter_context(tc.tile_pool(name="big", bufs=1))
            scanp = ctx.enter_context(tc.tile_pool(name="scan", bufs=4))
            spool = ctx.enter_context(tc.tile_pool(name="sp", bufs=4))
            dpool = ctx.enter_context(tc.tile_pool(name="dp", bufs=3))
            hcp = ctx.enter_context(tc.tile_pool(name="hc", bufs=2))
            accp = ctx.enter_context(tc.tile_pool(name="acc", bufs=2))
            chk = ctx.enter_context(tc.tile_pool(name="chk", bufs=3))
            ps = ctx.enter_context(tc.tile_pool(name="ps", bufs=2, space="PSUM"))
            psf1 = ctx.enter_context(tc.tile_pool(name="psf1", bufs=2, space="PSUM"))

            # ---- load constants ----
            # conv7-critical tensors first so the PE can start ASAP
            cc = {}
            xint = bigp.tile([C, 2 * P7 * P7], FP8, tag="pad")
            nc.sync.dma_start(xint[:], din["xint"])
            for name in ["wdr7"]:
                ap = din[name]
                t = const.tile(list(ap.shape), ap.dtype, tag=name)
                nc.sync.dma_start(t[:], ap)
                cc[name] = t
            for name, ap in din.items():
                if name in ("xint", "xres") or name in cc:
                    continue
                t = const.tile(list(ap.shape), ap.dtype, tag=name)
                nc.sync.dma_start(t[:], ap)
                cc[name] = t
            # Route the bias bank through an ACT copy: the ACT instruction
            # encoding has a single sync-wait slot, so later ACT ops must not
            # need a DMA wait on top of their PSUM wait.
            nv = len(ix)
            vecs_sb = const.tile([128, nv], F32, tag="vecs_sb")
            nc.scalar.activation(vecs_sb[:], cc["vecs"][:], AF.Copy)
            # dummy reader absorbs the same-engine RAW wait on vecs_sb so
            # later ACT ops keep a single wait slot for their PSUM input
            scr = const.tile([128, 1], F32, tag="scr")
            nc.scalar.activation(scr[:], vecs_sb[:, 0:1], AF.Copy)
            V = lambda key: vecs_sb[:, ix[key]:ix[key] + 1]
            V96 = lambda key: vecs_sb[:C, ix[key]:ix[key] + 1]

            xres = bigp.tile([C, L], F32, tag="xres")
            nc.sync.dma_start(xres[:].bitcast(F32R), din["xres"])

            # =============== helpers ===============
            def dwconv_psum(src_pad, Wp, ktaps, diag_off, jchunk, ptag):
                """depthwise conv for output rows [8*j, 8*j+8) -> psum [96,512]"""
                taps, rr = _taps(ktaps)
                pt = ps.tile([C, CH], F32, tag=ptag)
                src3 = src_pad[:].rearrange("c (h w) -> c h w", w=Wp)
                r0 = jchunk * 8
                nt = len(taps)
                for t, (dh, dw) in enumerate(taps):
                    rhs = src3[:, r0 + dh:r0 + dh + 8, dw:dw + W]
                    nc.tensor.matmul(
                        pt[:],
                        cc["wdiag"][:, (diag_off + t) * C:(diag_off + t + 1) * C],
                        rhs,
                        start=(t == 0),
                        stop=(t == nt - 1),
                    )
                return pt

            def ln_norm_chunk(src_chunk, src_f32, out_chunk, sq_gp=False):
                """out = (x - mean_c) / sqrt(var_c + eps) for one 512-chunk.

                Fused partition reduce+broadcast via all-ones [96,96] lhsT.
                sq_gp: compute the square on GPSIMD (for phases where ACT is
                the busy engine is false / DVE busy).
                """
                mb = ps.tile([C, CH], F32, tag="gen")
                if src_f32:
                    nc.tensor.matmul(mb[:], cc["ones96"][:], r32(src_chunk),
                                     start=True, stop=True)
                else:
                    nc.tensor.matmul(mb[:], cc["ones96_bf"][:], src_chunk,
                                     start=True, stop=True)
                d = chk.tile([C, CH], BF16, tag="lnd")
                nc.vector.scalar_tensor_tensor(d[:], mb[:], -1.0 / C, src_chunk,
                                               OP.mult, OP.add)
                dsq = chk.tile([C, CH], BF16, tag="sq")
                if sq_gp:
                    nc.gpsimd.tensor_tensor(dsq[:], d[:], d[:], OP.mult)
                else:
                    nc.scalar.activation(dsq[:], d[:], AF.Square)
                vb = ps.tile([C, CH], F32, tag="gen2")
                nc.tensor.matmul(vb[:], cc["ones96_bf"][:], dsq[:],
                                 start=True, stop=True)
                rstd = chk.tile([C, CH], BF16, tag="rstd")
                ph.tag(nc.scalar.activation(rstd[:], vb[:],
                                            AF.Abs_reciprocal_sqrt,
                                            scale=1.0 / C, bias=V96("eps")),
                       "rsqrt")
                nc.vector.tensor_tensor(out_chunk, d[:], rstd[:], OP.mult)

            def mlp_block(src_tile, src_f32, wf1, wf2, b1pfx, b2key, res_tile,
                          out_tile, round_out=False, sq_gp=False):
                """out = res + fc2(gelu(fc1(LN(src)))) ; all chunked.

                LN chunks are materialized fully first so the rsqrt ACT
                table phase doesn't interleave with the gelu table phase.
                """
                xnf = scanp.tile([C, L], BF16, tag="sc", name="xnf")
                for j in range(NCHUNK):
                    ln_norm_chunk(src_tile[:, j * CH:(j + 1) * CH], src_f32,
                                  xnf[:, j * CH:(j + 1) * CH], sq_gp=True)
                mlp_fc(xnf, wf1, wf2, b1pfx, b2key, res_tile, out_tile,
                       round_out)

            def mlp_fc(xnf, wf1, wf2, b1pfx, b2key, res_tile, out_tile,
                       round_out=False):
                for j in range(NCHUNK):
                    xn = xnf[:, j * CH:(j + 1) * CH]
                    gs = []
                    for mm in range(3):
                        f1 = psf1.tile([128, CH], F32, tag="f1")
                        nc.tensor.matmul(f1[:], cc[wf1][:, mm * 128:(mm + 1) * 128],
                                         xn, start=True, stop=True)
                        g = chk.tile([128, CH], BF16, tag=f"g{mm}")
                        ph.tag(nc.scalar.activation(g[:], f1[:], AF.Gelu,
                                                    bias=V(f"{b1pfx}{mm}")),
                               "gelu")
                        gs.append(g)
                    f2 = ps.tile([C, CH], F32, tag="gen")
                    for mm in range(3):
                        nc.tensor.matmul(f2[:], cc[wf2][:, mm * C:(mm + 1) * C],
                                         gs[mm][:], start=(mm == 0), stop=(mm == 2))
                    oap = out_tile[:, j * CH:(j + 1) * CH]
                    if round_out:
                        oap = oap.bitcast(F32R)
                    nc.vector.scalar_tensor_tensor(
                        oap, f2[:], V96(b2key),
                        res_tile[:, j * CH:(j + 1) * CH], OP.add, OP.add)

            # =============== ConvNeXt block: 7x7 dwconv (fp8 DoubleRow) ====
            # xint4[c, h, w, t]: t=0 is pixel (h,w), t=1 is pixel (h,w+1)
            # LN-stats work for chunk j-1 is emitted inside the conv loop so
            # the scheduler interleaves PE stats matmuls with conv matmuls
            # and ACT/DVE ramp up during the conv instead of after it.
            xint4 = xint[:].rearrange("c (h w t) -> c h w t", w=P7, t=2)
            wdr7v = cc["wdr7"][:].rearrange("c (p t m) -> c p t m", t=2, m=C)
            hsb = bigp.tile([C, L], BF16, tag="bufA")
            xnf_cn = scanp.tile([C, L], BF16, tag="sc", name="xnf_cn")
            for j in range(NCHUNK):
                pt = ps.tile([C, CH], F32, tag="cv")
                r0 = j * 8
                for dh in range(7):
                    for jw in range(4):
                        pr = dh * 4 + jw
                        rhs = xint4[:, r0 + dh:r0 + dh + 8,
                                    2 * jw:2 * jw + W, :].transpose([0, 3, 1, 2])
                        nc.tensor.matmul(pt[:], wdr7v[:, pr], rhs,
                                         start=(pr == 0), stop=(pr == 27),
                                         perf_mode=PM.DoubleRow)
                nc.scalar.activation(hsb[:, j * CH:(j + 1) * CH], pt[:],
                                     AF.Identity, scale=1.0 / CSCALE,
                                     bias=V96("cn_dw_b"))
                if j >= 1:
                    ln_norm_chunk(hsb[:, (j - 1) * CH:j * CH], False,
                                  xnf_cn[:, (j - 1) * CH:j * CH], sq_gp=True)
            ln_norm_chunk(hsb[:, 7 * CH:8 * CH], False,
                          xnf_cn[:, 7 * CH:8 * CH], sq_gp=True)
            # ConvNeXt output written in-place into the residual tile
            # (elementwise, same-index read+write)
            x1 = xres
            mlp_fc(xnf_cn, "wfc1", "wfc2", "fc1b", "fc2b", xres, x1,
                   round_out=True)

            # =============== SS2D: LN1 + in_proj + dwconv3 + silu ==========
            v2pad = bigp.tile([C, P3 * P3], BF16, tag="pad")
            nc.gpsimd.memset(v2pad[:], 0.0)
            v2int = v2pad[:].rearrange("c (h w) -> c h w", w=P3)
            xn1f = scanp.tile([C, L], BF16, tag="sc", name="xn1f")
            for j in range(NCHUNK):
                ln_norm_chunk(x1[:, j * CH:(j + 1) * CH], True,
                              xn1f[:, j * CH:(j + 1) * CH], sq_gp=True)
            v4 = bigp.tile([C, L], BF16, tag="bufA")

            def conv3_chunk(j):
                pc = dwconv_psum(v2pad, P3, 3, 0, j, "cv")
                # silu via the sigmoid table set (same set as the scan's
                # sigmoids -> no extra table load, no phase barrier between
                # the conv3 epilogue and the scan projections); the multiply
                # runs on the otherwise-idle DVE
                s8 = chk.tile([C, CH], BF16, tag="sq")
                ph.tag(nc.scalar.activation(s8[:], pc[:], AF.Sigmoid,
                                            bias=V96("dwb")), "sig")
                xb = chk.tile([C, CH], BF16, tag="lnd")
                nc.vector.tensor_scalar(xb[:], pc[:], 1.0, V96("dwb"),
                                        OP.mult, OP.add)
                nc.vector.tensor_tensor(v4[:, j * CH:(j + 1) * CH], xb[:],
                                        s8[:], OP.mult)

            # in_proj bias epilogue on DVE (tensor_scalar: psum*1 + ipb) —
            # DVE idles in this window while ACT is the busy engine; conv3
            # chunk j-2 needs only in_proj rows <= 8(j-1)+9, so interleave
            # its emission to spread the PE work
            for j in range(NCHUNK):
                pv = ps.tile([C, CH], F32, tag="gen")
                nc.tensor.matmul(pv[:], cc["wip"][:],
                                 xn1f[:, j * CH:(j + 1) * CH], start=True,
                                 stop=True)
                dst = v2int[:, 1 + j * 8:1 + (j + 1) * 8, 1:1 + W]
                nc.vector.tensor_scalar(dst, pv[:], 1.0, V96("ipb"),
                                        OP.mult, OP.add)
                if j >= 2:
                    conv3_chunk(j - 2)
            for j in (6, 7):
                conv3_chunk(j)

            # =============== per-direction scan ===============
            # v4T: w-major view for the transposed directions (k=1,3)
            v4T = v4[:].rearrange("c (h w) -> c h w", w=W).transpose([0, 2, 1])

            def xrhs(k, j):
                if k in (0, 2):
                    return v4[:, j * CH:(j + 1) * CH]
                return v4T[:, j * 8:(j + 1) * 8, :]

            # s = sigmoid(-(z + dtb)) per chunk; delta' = ln(s) = -softplus(z)
            # (sign folded into negated B weights); dA = s when A == -1.
            S = {}
            D = {}
            for pair in ([0, 2], [1, 3]):
                for k in pair:
                    S[k] = spool.tile([C, L], BF16, tag="S", name=f"S{k}")
                    for j in range(NCHUNK):
                        pd = ps.tile([C, CH], F32, tag="gen")
                        nc.tensor.matmul(pd[:],
                                         cc["wdt2"][:, k * C:(k + 1) * C],
                                         xrhs(k, j), start=True, stop=True)
                        ph.tag(nc.scalar.activation(
                            S[k][:, j * CH:(j + 1) * CH], pd[:], AF.Sigmoid,
                            scale=-1.0, bias=V96(f"dtbn{k}")), "sig")
                for k in pair:
                    D[k] = dpool.tile([C, L], BF16, tag="D", name=f"D{k}")
                    ph.tag(nc.scalar.activation(D[k][:], S[k][:], AF.Ln), "ln")

            accs = {}
            for k in [0, 2, 1, 3]:
                # dA: sigmoid tile directly (A == -1), else exp(-A * delta')
                if a_is_neg1:
                    dA = S[k]
                else:
                    dA = scanp.tile([C, L], BF16, tag="sc", name=f"dA{k}")
                    ph.tag(nc.scalar.activation(dA[:], D[k][:], AF.Exp,
                                                scale=V96(f"An{k}")), "exp")
                bso = scanp.tile([C, L], BF16, tag="sc")
                for j in range(NCHUNK):
                    bb = ps.tile([C, CH], F32, tag="gen2")
                    nc.tensor.matmul(bb[:],
                                     cc["wbc"][:, (2 * k) * C:(2 * k + 1) * C],
                                     xrhs(k, j), start=True, stop=True)
                    du = chk.tile([C, CH], BF16, tag="du")
                    if k in (0, 2):
                        nc.vector.tensor_tensor(du[:],
                                                D[k][:, j * CH:(j + 1) * CH],
                                                xrhs(k, j), OP.mult)
                        # k0/k2 run during the sigmoid ACT phase: read the
                        # B broadcast straight from PSUM on DVE
                        nc.vector.tensor_tensor(bso[:, j * CH:(j + 1) * CH],
                                                du[:], bb[:], OP.mult)
                    else:
                        nc.gpsimd.tensor_tensor(du[:],
                                                D[k][:, j * CH:(j + 1) * CH],
                                                xrhs(k, j), OP.mult)
                        # k1/k3 run after the sigmoid phase: ACT has slack,
                        # stage B via ACT so the DVE multiply runs 2x bf16
                        bbs = chk.tile([C, CH], BF16, tag="bb")
                        nc.scalar.activation(bbs[:], bb[:], AF.Identity)
                        nc.vector.tensor_tensor(bso[:, j * CH:(j + 1) * CH],
                                                du[:], bbs[:], OP.mult)
                h = scanp.tile([C, L], BF16, tag="sc")
                HL = L // 2
                if k in (0, 1):
                    # split scan with state carry: half 1 starts after only
                    # half the bso chunks; y-multiplies of the first half
                    # don't wait for the second
                    nc.vector.tensor_tensor_scan(h[:, 0:HL], dA[:, 0:HL],
                                                 bso[:, 0:HL], 0.0,
                                                 OP.mult, OP.add)
                    nc.vector.tensor_tensor_scan(h[:, HL:L], dA[:, HL:L],
                                                 bso[:, HL:L],
                                                 h[:, HL - 1:HL],
                                                 OP.mult, OP.add)
                else:
                    # reversed scan: scan positions 0..HL-1 are cols L-1..HL
                    nc.vector.tensor_tensor_scan(h[:, HL:L][:, ::-1],
                                                 dA[:, HL:L][:, ::-1],
                                                 bso[:, HL:L][:, ::-1], 0.0,
                                                 OP.mult, OP.add)
                    nc.vector.tensor_tensor_scan(h[:, 0:HL][:, ::-1],
                                                 dA[:, 0:HL][:, ::-1],
                                                 bso[:, 0:HL][:, ::-1],
                                                 h[:, HL:HL + 1],
                                                 OP.mult, OP.add)
                # y_k = h * Cs_b  (+ accumulate into l-major / w-major accs)
                if k in (0, 1):
                    dst = hcp.tile([C, L], BF16, tag="hc", name=f"hc{k}")
                else:
                    dst = accp.tile([C, L], BF16, tag="acc", name=f"acc{k}")
                for j in range(NCHUNK):
                    cb = ps.tile([C, CH], F32, tag="gen2")
                    nc.tensor.matmul(cb[:],
                                     cc["wbc"][:, (2 * k + 1) * C:(2 * k + 2) * C],
                                     xrhs(k, j), start=True, stop=True)
                    # stage C broadcast through ACT (idle in this phase) so
                    # the DVE multiplies run in 2x bf16 mode
                    cbs = chk.tile([C, CH], BF16, tag="bb")
                    nc.scalar.activation(cbs[:], cb[:], AF.Identity)
                    if k in (0, 1):
                        nc.vector.tensor_tensor(dst[:, j * CH:(j + 1) * CH],
                                                h[:, j * CH:(j + 1) * CH],
                                                cbs[:], OP.mult)
                    else:
                        tmp = chk.tile([C, CH], BF16, tag="du")
                        nc.vector.tensor_tensor(tmp[:],
                                                h[:, j * CH:(j + 1) * CH],
                                                cbs[:], OP.mult)
                        nc.vector.tensor_tensor(dst[:, j * CH:(j + 1) * CH],
                                                accs[k - 2][:,
                                                            j * CH:(j + 1) * CH],
                                                tmp[:], OP.add)
                accs[k] = dst

            # =============== cross-merge + D*u + LN + out_proj =============
            preln = hcp.tile([C, L], BF16, tag="hc")
            accT = accs[3][:].rearrange("c (w h) -> c w h", w=W).transpose(
                [0, 2, 1])
            for j in range(NCHUNK):
                t2 = chk.tile([C, CH], BF16, tag="du")
                nc.gpsimd.tensor_tensor(t2[:],
                                        accs[2][:, j * CH:(j + 1) * CH],
                                        accT[:, j * 8:(j + 1) * 8, :], OP.add)
                nc.vector.scalar_tensor_tensor(
                    preln[:, j * CH:(j + 1) * CH],
                    v4[:, j * CH:(j + 1) * CH], V96("Dsum"), t2[:],
                    OP.mult, OP.add)
            x2 = bigp.tile([C, L], F32, tag="x2")
            ynf = scanp.tile([C, L], BF16, tag="sc", name="ynf")
            for j in range(NCHUNK):
                ln_norm_chunk(preln[:, j * CH:(j + 1) * CH], False,
                              ynf[:, j * CH:(j + 1) * CH], sq_gp=True)
            for j in range(NCHUNK):
                po = ps.tile([C, CH], F32, tag="gen")
                nc.tensor.matmul(po[:], cc["wop"][:],
                                 ynf[:, j * CH:(j + 1) * CH], start=True,
                                 stop=True)
                nc.vector.scalar_tensor_tensor(x2[:, j * CH:(j + 1) * CH]
                                               .bitcast(F32R), po[:],
                                               V96("opb"),
                                               x1[:, j * CH:(j + 1) * CH],
                                               OP.add, OP.add)

            # =============== MLP block ===============
            outsb = bigp.tile([C, L], F32, tag="xres")
            mlp_block(x2, True, "wmfc1", "wmfc2", "mfc1b", "mfc2b", x2, outsb,
                      sq_gp=True)
            nc.sync.dma_start(dout, outsb[:])

    nc.compile()
    return nc


def get_program_and_inputs(inputs):
    host, ix, a_is_neg1 = build_host_tensors(inputs)
    key = ("prog", a_is_neg1)
    if key not in _CACHE:
        _CACHE[key] = build_program(ix, a_is_neg1)
    nc = _CACHE[key]
    x = np.asarray(inputs["x"], np.float32)
    in_maps = []
    for b in range(B):
        m = {k: v for k, v in host.items()}
        m["xint"] = pad_image(x[b])
        # pre-round to the 2xbf16 (fp32r) representable set: this tile is
        # read by fp32r LN-stats matmuls before being overwritten in place
        xr = x[b].reshape(C, L).astype(np.float32)
        hi = xr.astype(bfnp).astype(np.float32)
        lo = (xr - hi).astype(bfnp).astype(np.float32)
        m["xres"] = hi + lo
        in_maps.append(m)
    return nc, in_maps


def kernel(**inputs):
    nc, in_maps = get_program_and_inputs(inputs)
    res = run_bass_kernel_spmd(nc, in_maps, list(range(B)))
    out = np.stack([res.results[b]["out"].reshape(C, H, W) for b in range(B)])
    return out.astype(np.float32)


if __name__ == "__main__":
    # smoke build
    host, ix, a_neg1 = build_host_tensors(
        {k: np.zeros(s, np.float32) for k, s in [  # noqa

            ("x", (B, C, H, W)), ("cn_dw_w", (C, 7, 7)), ("cn_dw_b", (C,)),
            ("cn_ln_w", (C,)), ("cn_ln_b", (C,)), ("cn_fc1_w", (4 * C, C)),
            ("cn_fc1_b", (4 * C,)), ("cn_fc2_w", (C, 4 * C)), ("cn_fc2_b", (C,)),
            ("v_ln1_w", (C,)), ("v_ln1_b", (C,)), ("ip_w", (C, C)),
            ("ip_b", (C,)), ("dw_w", (C, 3, 3)), ("dw_b", (C,)),
            ("x_proj_w", (K, R + 2 * N, C)), ("dt_w", (K, C, R)),
            ("dt_b", (K, C)), ("A_logs", (K * C, N)), ("Ds", (K * C,)),
            ("o_ln_w", (C,)), ("o_ln_b", (C,)), ("op_w", (C, C)),
            ("op_b", (C,)), ("v_ln2_w", (C,)), ("v_ln2_b", (C,)),
            ("m_fc1_w", (4 * C, C)), ("m_fc1_b", (4 * C,)),
            ("m_fc2_w", (C, 4 * C)), ("m_fc2_b", (C,)),
        ]})
    nc = build_program(ix, a_neg1)
    print("program built OK:", len(list(nc.all_instructions())), "instructions")


# revision 56
# speedup vs baseline: 1.0310x; 1.0310x over previous
"""ConvVMamba TRN2 Bass kernel.

Sharding: data-parallel over batch. B=8 -> one image per NeuronCore, all
weights replicated, no collectives.

Per-core layout: channels on SBUF partitions (C=96), pixels on the free dim
(L=64*64=4096).
  - 1x1 convs / projections: PE matmuls (lhsT = W^T, rhs = activations).
  - depthwise 7x7 / 3x3: PE accumulating matmuls with per-tap diagonal lhsT
    over a zero-padded image buffer.
  - LayerNorm over channels: partition reduction via ones-vector matmul into
    [96,512] stats, Rsqrt ACT for the inverse std, ones-matmul broadcast.
  - selective scan (d_state=1): DVE tensor_tensor_scan (state = a*state + b),
    reversed directions via negative-step APs, transposed directions by
    keeping the whole per-direction pipeline in w-major order.
  - dt/B/C projections fused host-side into per-direction [96,96] lhsT
    acting directly on v4 (no xdbl intermediate).
  - softplus via sigmoid+ln: s = sigmoid(-z), delta' = ln(s) = -softplus(z);
    the sign is folded into the B weights; when A == -1 (d_state=1 init)
    dA = exp(A*softplus(z)) = s, so the scan coefficient is free.
  - ACT function-table discipline: only {rsqrt, gelu, silu, sigmoid, ln,
    exp} need table sets; ops are emitted (and fenced) in set-contiguous
    phases so only ~9 table loads happen instead of one per op.
Branch tensors in bf16, residual stream in fp32.
"""

import sys
import numpy as np

sys.path.insert(0, "/opt/trn_rl_repo")

import ml_dtypes  # noqa: E402
import concourse.bass as bass  # noqa: E402
import concourse.bacc as bacc  # noqa: E402
import concourse.mybir as mybir  # noqa: E402
from concourse import tile  # noqa: E402
from concourse.tile import add_dep_helper  # noqa: E402
from concourse.bass_utils import run_bass_kernel_spmd  # noqa: E402

F32 = mybir.dt.float32
F32R = mybir.dt.float32r
BF16 = mybir.dt.bfloat16
FP8 = mybir.dt.float8e4
PM = mybir.MatmulPerfMode
AF = mybir.ActivationFunctionType
OP = mybir.AluOpType
bfnp = ml_dtypes.bfloat16
f8np = ml_dtypes.float8_e4m3fn
CSCALE = 2048.0  # conv7 fp8 weight scale (power of 2; dequant in bias ACT)

B, C, H, W = 8, 96, 64, 64
L = H * W
R, N, K = 6, 1, 4
EPS = 1e-5
P7, P3 = 70, 66  # padded widths for 7x7 and 3x3 convs
NCHUNK = 8  # 4096 / 512
CH = 512

_CACHE = {}


def _taps(k):
    r = (k - 1) // 2
    return [(dh, dw) for dh in range(k) for dw in range(k)], r


def build_host_tensors(kw):
    """Precompute all weight/constant DRAM tensors (shared across cores)."""
    f = lambda a: np.asarray(a, np.float32)
    out = {}

    # --- fold LN gamma/beta into following 1x1 convs ---
    def fold(wname, bname, g, b):
        w = f(kw[wname])
        bb = f(kw[bname])
        return w * f(g)[None, :], bb + w @ f(b)

    fc1w, fc1b = fold("cn_fc1_w", "cn_fc1_b", kw["cn_ln_w"], kw["cn_ln_b"])
    ipw, ipb = fold("ip_w", "ip_b", kw["v_ln1_w"], kw["v_ln1_b"])
    opw, opb = fold("op_w", "op_b", kw["o_ln_w"], kw["o_ln_b"])
    mfc1w, mfc1b = fold("m_fc1_w", "m_fc1_b", kw["v_ln2_w"], kw["v_ln2_b"])
    fc2w, fc2b = f(kw["cn_fc2_w"]), f(kw["cn_fc2_b"])
    mfc2w, mfc2b = f(kw["m_fc2_w"]), f(kw["m_fc2_b"])

    # --- depthwise 7x7: fp8 DoubleRow pair weights [96, 28*(2*96)] ---
    # pair pr = dh*4 + jw covers taps (dh, 2jw) and (dh, 2jw+1); dw=7 is a
    # phantom zero tap. Weights scaled by CSCALE for fp8 range; the conv
    # output ACT applies 1/CSCALE.
    w7 = f(kw["cn_dw_w"])  # [C, 7, 7]
    ar = np.arange(C)
    wdr7 = np.zeros((C, 28, 2, C), np.float32)
    for dh in range(7):
        for jw in range(4):
            pr = dh * 4 + jw
            wdr7[ar, pr, 0, ar] = w7[:, dh, 2 * jw] * CSCALE
            if 2 * jw + 1 < 7:
                wdr7[ar, pr, 1, ar] = w7[:, dh, 2 * jw + 1] * CSCALE
    out["wdr7"] = wdr7.reshape(C, 28 * 2 * C).astype(f8np)

    # --- depthwise 3x3 diagonals: [96, 9*96] bf16 ---
    w3 = f(kw["dw_w"]).reshape(C, 9)
    diag = np.zeros((C, 9 * C), np.float32)
    for t in range(9):
        diag[ar, t * C + ar] = w3[:, t]
    out["wdiag"] = diag.astype(bfnp)

    # --- GEMM weights (lhsT layouts), bf16 ---
    out["wfc1"] = fc1w.T.astype(bfnp)  # [96, 384]
    wfc2 = np.zeros((128, 3 * C), np.float32)  # [128, 288] K-chunks
    for j in range(3):
        wfc2[:, j * C:(j + 1) * C] = fc2w[:, j * 128:(j + 1) * 128].T
    out["wfc2"] = wfc2.astype(bfnp)
    out["wip"] = ipw.T.astype(bfnp)  # [96, 96]
    out["wop"] = opw.T.astype(bfnp)
    out["wmfc1"] = mfc1w.T.astype(bfnp)
    wm2 = np.zeros((128, 3 * C), np.float32)
    for j in range(3):
        wm2[:, j * C:(j + 1) * C] = mfc2w[:, j * 128:(j + 1) * 128].T
    out["wmfc2"] = wm2.astype(bfnp)

    # --- fused per-direction dt projection and B/C broadcast lhsT ---
    # delta_raw_k = (dt_w[k] @ x_proj_w[k][:R]) @ v4  -> lhsT [96, 96]
    # B_bcast_k[m,p] = x_proj_w[k][R] . v4[:,p]  (negated: softplus sign fold)
    # C_bcast_k[m,p] = x_proj_w[k][R+N] . v4[:,p]
    xp = f(kw["x_proj_w"])  # [4, 8, 96]
    dtw = f(kw["dt_w"])  # [4, 96, 6]
    wdt2 = np.zeros((C, 4 * C), np.float32)
    wbc = np.zeros((C, 8 * C), np.float32)
    for k in range(4):
        wdt2[:, k * C:(k + 1) * C] = (dtw[k] @ xp[k][:R]).T
        wbc[:, (2 * k) * C:(2 * k + 1) * C] = np.repeat(
            -xp[k][R][:, None], C, axis=1)
        wbc[:, (2 * k + 1) * C:(2 * k + 2) * C] = np.repeat(
            xp[k][R + N][:, None], C, axis=1)
    out["wdt2"] = wdt2.astype(bfnp)
    out["wbc"] = wbc.astype(bfnp)

    # ones for LN partition reduce+broadcast
    out["ones96"] = np.ones((C, C), np.float32)
    out["ones96_bf"] = np.ones((C, C), bfnp)

    # per-partition scalar bank [128, NV] fp32
    A = (-np.exp(f(kw["A_logs"]))).reshape(K, C)
    Ds = f(kw["Ds"]).reshape(K, C)
    dtb = f(kw["dt_b"])  # [4, 96]
    cols = []

    def col(v, n=C):
        a = np.zeros(128, np.float32)
        a[: len(v)] = v
        cols.append(a)
        return len(cols) - 1

    ix = {}
    ix["cn_dw_b"] = col(f(kw["cn_dw_b"]))
    for j in range(3):
        ix[f"fc1b{j}"] = col(fc1b[j * 128:(j + 1) * 128])
    ix["fc2b"] = col(fc2b)
    ix["ipb"] = col(ipb)
    ix["dwb"] = col(f(kw["dw_b"]))
    for k in range(4):
        ix[f"dtbn{k}"] = col(-dtb[k])   # sigmoid(-(z+dtb)) bias
        ix[f"An{k}"] = col(-A[k])       # general path: dA = exp(-A * delta')
    ix["Dsum"] = col(Ds.sum(0))
    ix["eps"] = col(np.full(128, EPS, np.float32), 128)
    ix["opb"] = col(opb)
    for j in range(3):
        ix[f"mfc1b{j}"] = col(mfc1b[j * 128:(j + 1) * 128])
    ix["mfc2b"] = col(mfc2b)
    out["vecs"] = np.stack(cols, axis=1)  # [128, NV]
    a_is_neg1 = bool(np.allclose(A, -1.0, atol=0.0, rtol=0.0))
    return out, ix, a_is_neg1


def pad_image(x):
    """[96,64,64] fp32 -> dup-interleaved fp8 [96, 2*70*70].

    xint[c, 2q] = xpad[c, q]; xint[c, 2q+1] = xpad[c, q+1] so any
    horizontally-adjacent tap pair is a 2-byte-aligned fp8 pair for
    DoubleRow matmuls.
    """
    xp = np.zeros((C, P7, P7), np.float32)
    xp[:, 3:3 + H, 3:3 + W] = x
    flat = np.zeros((C, P7 * P7 + 1), np.float32)
    flat[:, :P7 * P7] = xp.reshape(C, P7 * P7)
    xint = np.empty((C, 2 * P7 * P7), np.float32)
    xint[:, 0::2] = flat[:, :-1]
    xint[:, 1::2] = flat[:, 1:]
    return xint.astype(f8np)


def r32(ap):
    return ap.bitcast(F32R)


def build_program(ix, a_is_neg1):
    nc = bacc.Bacc("TRN2", target_bir_lowering=False, debug=False)

    din = {}
    for name, shape, dt in [
        ("xint", [C, 2 * P7 * P7], FP8),
        ("xres", [C, L], F32R),
        ("wdr7", [C, 28 * 2 * C], FP8),
        ("wdiag", [C, 9 * C], BF16),
        ("wfc1", [C, 384], BF16),
        ("wfc2", [128, 3 * C], BF16),
        ("wip", [C, C], BF16),
        ("wop", [C, C], BF16),
        ("wmfc1", [C, 384], BF16),
        ("wmfc2", [128, 3 * C], BF16),
        ("wdt2", [C, 4 * C], BF16),
        ("wbc", [C, 8 * C], BF16),
        ("ones96", [C, C], F32R),
        ("ones96_bf", [C, C], BF16),
        ("vecs", [128, len(ix)], F32),
    ]:
        din[name] = nc.dram_tensor(name, shape, dt, kind="ExternalInput").ap()
    dout = nc.dram_tensor("out", [C, L], F32, kind="ExternalOutput").ap()

    class ActPhase:
        # Fences set-specific ACT ops so the scheduler cannot interleave
        # ops from different activation-table sets (each flip costs an ACT
        # function-table reload). Phase-barrier semantics: each op depends
        # on every op of the PREVIOUS phase, but ops within one phase are
        # unordered so the scheduler can run them by data readiness.
        def __init__(self):
            self.prev_phase = []
            self.cur_phase = []
            self.cur_set = None

        def tag(self, bi, fset):
            inst = bi.ins
            if fset != self.cur_set:
                self.prev_phase = self.cur_phase
                self.cur_phase = []
                self.cur_set = fset
            for p in self.prev_phase:
                add_dep_helper(inst, p, sync=True,
                               reason="act table-set phase fence")
            self.cur_phase.append(inst)
            return bi

    ph = ActPhase()

    with tile.TileContext(nc) as tc:
        from contextlib import ExitStack

        with ExitStack() as ctx:
            const = ctx.enter_context(tc.tile_pool(name="const", bufs=1))
            bigp = ctx.enter_context(tc.tile_pool(name="big", bufs=1))
            scanp = ctx.enter_context(tc.tile_pool(name="scan", bufs=4))
            spool = ctx.enter_context(tc.tile_pool(name="sp", bufs=4))
            dpool = ctx.enter_context(tc.tile_pool(name="dp", bufs=3))
            hcp = ctx.enter_context(tc.tile_pool(name="hc", bufs=2))
            accp = ctx.enter_context(tc.tile_pool(name="acc", bufs=2))
            chk = ctx.enter_context(tc.tile_pool(name="chk", bufs=3))
            ps = ctx.enter_context(tc.tile_pool(name="ps", bufs=2, space="PSUM"))
            psf1 = ctx.enter_context(tc.tile_pool(name="psf1", bufs=2, space="PSUM"))

            # ---- load constants ----
            # conv7-critical tensors first so the PE can start ASAP
            cc = {}
            xint = bigp.tile([C, 2 * P7 * P7], FP8, tag="pad")
            nc.sync.dma_start(xint[:], din["xint"])
            for name in ["wdr7"]:
                ap = din[name]
                t = const.tile(list(ap.shape), ap.dtype, tag=name)
                nc.sync.dma_start(t[:], ap)
                cc[name] = t
            for name, ap in din.items():
                if name in ("xint", "xres") or name in cc:
                    continue
                t = const.tile(list(ap.shape), ap.dtype, tag=name)
                nc.sync.dma_start(t[:], ap)
                cc[name] = t
            # Route the bias bank through an ACT copy: the ACT instruction
            # encoding has a single sync-wait slot, so later ACT ops must not
            # need a DMA wait on top of their PSUM wait.
            nv = len(ix)
            vecs_sb = const.tile([128, nv], F32, tag="vecs_sb")
            nc.scalar.activation(vecs_sb[:], cc["vecs"][:], AF.Copy)
            # dummy reader absorbs the same-engine RAW wait on vecs_sb so
            # later ACT ops keep a single wait slot for their PSUM input
            scr = const.tile([128, 1], F32, tag="scr")
            nc.scalar.activation(scr[:], vecs_sb[:, 0:1], AF.Copy)
            V = lambda key: vecs_sb[:, ix[key]:ix[key] + 1]
            V96 = lambda key: vecs_sb[:C, ix[key]:ix[key] + 1]

            xres = bigp.tile([C, L], F32, tag="xres")
            nc.sync.dma_start(xres[:].bitcast(F32R), din["xres"])

            # =============== helpers ===============
            def dwconv_psum(src_pad, Wp, ktaps, diag_off, jchunk, ptag):
                """depthwise conv for output rows [8*j, 8*j+8) -> psum [96,512]"""
                taps, rr = _taps(ktaps)
                pt = ps.tile([C, CH], F32, tag=ptag)
                src3 = src_pad[:].rearrange("c (h w) -> c h w", w=Wp)
                r0 = jchunk * 8
                nt = len(taps)
                for t, (dh, dw) in enumerate(taps):
                    rhs = src3[:, r0 + dh:r0 + dh + 8, dw:dw + W]
                    nc.tensor.matmul(
                        pt[:],
                        cc["wdiag"][:, (diag_off + t) * C:(diag_off + t + 1) * C],
                        rhs,
                        start=(t == 0),
                        stop=(t == nt - 1),
                    )
                return pt

            def ln_norm_chunk(src_chunk, src_f32, out_chunk, sq_gp=False):
                """out = (x - mean_c) / sqrt(var_c + eps) for one 512-chunk.

                Fused partition reduce+broadcast via all-ones [96,96] lhsT.
                sq_gp: compute the square on GPSIMD (for phases where ACT is
                the busy engine is false / DVE busy).
                """
                mb = ps.tile([C, CH], F32, tag="gen")
                if src_f32:
                    nc.tensor.matmul(mb[:], cc["ones96"][:], r32(src_chunk),
                                     start=True, stop=True)
                else:
                    nc.tensor.matmul(mb[:], cc["ones96_bf"][:], src_chunk,
                                     start=True, stop=True)
                d = chk.tile([C, CH], BF16, tag="lnd")
                nc.vector.scalar_tensor_tensor(d[:], mb[:], -1.0 / C, src_chunk,
                                               OP.mult, OP.add)
                dsq = chk.tile([C, CH], BF16, tag="sq")
                if sq_gp:
                    nc.gpsimd.tensor_tensor(dsq[:], d[:], d[:], OP.mult)
                else:
                    nc.scalar.activation(dsq[:], d[:], AF.Square)
                vb = ps.tile([C, CH], F32, tag="gen2")
                nc.tensor.matmul(vb[:], cc["ones96_bf"][:], dsq[:],
                                 start=True, stop=True)
                rstd = chk.tile([C, CH], BF16, tag="rstd")
                ph.tag(nc.scalar.activation(rstd[:], vb[:],
                                            AF.Abs_reciprocal_sqrt,
                                            scale=1.0 / C, bias=V96("eps")),
                       "rsqrt")
                nc.vector.tensor_tensor(out_chunk, d[:], rstd[:], OP.mult)

            def mlp_block(src_tile, src_f32, wf1, wf2, b1pfx, b2key, res_tile,
                          out_tile, round_out=False, sq_gp=False):
                """out = res + fc2(gelu(fc1(LN(src)))) ; all chunked.

                LN chunks are materialized fully first so the rsqrt ACT
                table phase doesn't interleave with the gelu table phase.
                """
                xnf = scanp.tile([C, L], BF16, tag="sc", name="xnf")
                for j in range(NCHUNK):
                    ln_norm_chunk(src_tile[:, j * CH:(j + 1) * CH], src_f32,
                                  xnf[:, j * CH:(j + 1) * CH], sq_gp=sq_gp)
                mlp_fc(xnf, wf1, wf2, b1pfx, b2key, res_tile, out_tile,
                       round_out)

            def mlp_fc(xnf, wf1, wf2, b1pfx, b2key, res_tile, out_tile,
                       round_out=False):
                for j in range(NCHUNK):
                    xn = xnf[:, j * CH:(j + 1) * CH]
                    gs = []
                    for mm in range(3):
                        f1 = psf1.tile([128, CH], F32, tag="f1")
                        nc.tensor.matmul(f1[:], cc[wf1][:, mm * 128:(mm + 1) * 128],
                                         xn, start=True, stop=True)
                        g = chk.tile([128, CH], BF16, tag=f"g{mm}")
                        ph.tag(nc.scalar.activation(g[:], f1[:], AF.Gelu,
                                                    bias=V(f"{b1pfx}{mm}")),
                               "gelu")
                        gs.append(g)
                    f2 = ps.tile([C, CH], F32, tag="gen")
                    for mm in range(3):
                        nc.tensor.matmul(f2[:], cc[wf2][:, mm * C:(mm + 1) * C],
                                         gs[mm][:], start=(mm == 0), stop=(mm == 2))
                    oap = out_tile[:, j * CH:(j + 1) * CH]
                    if round_out:
                        oap = oap.bitcast(F32R)
                    nc.vector.scalar_tensor_tensor(
                        oap, f2[:], V96(b2key),
                        res_tile[:, j * CH:(j + 1) * CH], OP.add, OP.add)

            # =============== ConvNeXt block: 7x7 dwconv (fp8 DoubleRow) ====
            # xint4[c, h, w, t]: t=0 is pixel (h,w), t=1 is pixel (h,w+1)
            # LN-stats work for chunk j-1 is emitted inside the conv loop so
            # the scheduler interleaves PE stats matmuls with conv matmuls
            # and ACT/DVE ramp up during the conv instead of after it.
            xint4 = xint[:].rearrange("c (h w t) -> c h w t", w=P7, t=2)
            wdr7v = cc["wdr7"][:].rearrange("c (p t m) -> c p t m", t=2, m=C)
            hsb = bigp.tile([C, L], BF16, tag="bufA")
            xnf_cn = scanp.tile([C, L], BF16, tag="sc", name="xnf_cn")
            for j in range(NCHUNK):
                pt = ps.tile([C, CH], F32, tag="cv")
                r0 = j * 8
                for dh in range(7):
                    for jw in range(4):
                        pr = dh * 4 + jw
                        rhs = xint4[:, r0 + dh:r0 + dh + 8,
                                    2 * jw:2 * jw + W, :].transpose([0, 3, 1, 2])
                        nc.tensor.matmul(pt[:], wdr7v[:, pr], rhs,
                                         start=(pr == 0), stop=(pr == 27),
                                         perf_mode=PM.DoubleRow)
                nc.scalar.activation(hsb[:, j * CH:(j + 1) * CH], pt[:],
                                     AF.Identity, scale=1.0 / CSCALE,
                                     bias=V96("cn_dw_b"))
                if j >= 1:
                    ln_norm_chunk(hsb[:, (j - 1) * CH:j * CH], False,
                                  xnf_cn[:, (j - 1) * CH:j * CH], sq_gp=False)
            ln_norm_chunk(hsb[:, 7 * CH:8 * CH], False,
                          xnf_cn[:, 7 * CH:8 * CH], sq_gp=False)
            # ConvNeXt output written in-place into the residual tile
            # (elementwise, same-index read+write)
            x1 = xres
            mlp_fc(xnf_cn, "wfc1", "wfc2", "fc1b", "fc2b", xres, x1,
                   round_out=True)

            # =============== SS2D: LN1 + in_proj + dwconv3 + silu ==========
            v2pad = bigp.tile([C, P3 * P3], BF16, tag="pad")
            nc.gpsimd.memset(v2pad[:], 0.0)
            v2int = v2pad[:].rearrange("c (h w) -> c h w", w=P3)
            xn1f = scanp.tile([C, L], BF16, tag="sc", name="xn1f")
            for j in range(NCHUNK):
                ln_norm_chunk(x1[:, j * CH:(j + 1) * CH], True,
                              xn1f[:, j * CH:(j + 1) * CH], sq_gp=False)
            v4 = bigp.tile([C, L], BF16, tag="bufA")

            def conv3_chunk(j):
                pc = dwconv_psum(v2pad, P3, 3, 0, j, "cv")
                # silu via the sigmoid table set (same set as the scan's
                # sigmoids -> no extra table load, no phase barrier between
                # the conv3 epilogue and the scan projections); the multiply
                # runs on the otherwise-idle DVE
                s8 = chk.tile([C, CH], BF16, tag="sq")
                ph.tag(nc.scalar.activation(s8[:], pc[:], AF.Sigmoid,
                                            bias=V96("dwb")), "sig")
                xb = chk.tile([C, CH], BF16, tag="lnd")
                nc.vector.tensor_scalar(xb[:], pc[:], 1.0, V96("dwb"),
                                        OP.mult, OP.add)
                nc.vector.tensor_tensor(v4[:, j * CH:(j + 1) * CH], xb[:],
                                        s8[:], OP.mult)

            # in_proj bias epilogue on DVE (tensor_scalar: psum*1 + ipb) —
            # DVE idles in this window while ACT is the busy engine; conv3
            # chunk j-2 needs only in_proj rows <= 8(j-1)+9, so interleave
            # its emission to spread the PE work
            for j in range(NCHUNK):
                pv = ps.tile([C, CH], F32, tag="gen")
                nc.tensor.matmul(pv[:], cc["wip"][:],
                                 xn1f[:, j * CH:(j + 1) * CH], start=True,
                                 stop=True)
                dst = v2int[:, 1 + j * 8:1 + (j + 1) * 8, 1:1 + W]
                nc.vector.tensor_scalar(dst, pv[:], 1.0, V96("ipb"),
                                        OP.mult, OP.add)
                if j >= 2:
                    conv3_chunk(j - 2)
            for j in (6, 7):
                conv3_chunk(j)

            # =============== per-direction scan ===============
            # v4T: w-major view for the transposed directions (k=1,3)
            v4T = v4[:].rearrange("c (h w) -> c h w", w=W).transpose([0, 2, 1])

            def xrhs(k, j):
                if k in (0, 2):
                    return v4[:, j * CH:(j + 1) * CH]
                return v4T[:, j * 8:(j + 1) * 8, :]

            # s = sigmoid(-(z + dtb)) per chunk; delta' = ln(s) = -softplus(z)
            # (sign folded into negated B weights); dA = s when A == -1.
            S = {}
            D = {}
            for pair in ([0, 2], [1, 3]):
                for k in pair:
                    S[k] = spool.tile([C, L], BF16, tag="S", name=f"S{k}")
                    for j in range(NCHUNK):
                        pd = ps.tile([C, CH], F32, tag="gen")
                        nc.tensor.matmul(pd[:],
                                         cc["wdt2"][:, k * C:(k + 1) * C],
                                         xrhs(k, j), start=True, stop=True)
                        ph.tag(nc.scalar.activation(
                            S[k][:, j * CH:(j + 1) * CH], pd[:], AF.Sigmoid,
                            scale=-1.0, bias=V96(f"dtbn{k}")), "sig")
                for k in pair:
                    D[k] = dpool.tile([C, L], BF16, tag="D", name=f"D{k}")
                    ph.tag(nc.scalar.activation(D[k][:], S[k][:], AF.Ln), "ln")

            accs = {}
            for k in [0, 2, 1, 3]:
                # dA: sigmoid tile directly (A == -1), else exp(-A * delta')
                if a_is_neg1:
                    dA = S[k]
                else:
                    dA = scanp.tile([C, L], BF16, tag="sc", name=f"dA{k}")
                    ph.tag(nc.scalar.activation(dA[:], D[k][:], AF.Exp,
                                                scale=V96(f"An{k}")), "exp")
                bso = scanp.tile([C, L], BF16, tag="sc")
                for j in range(NCHUNK):
                    bb = ps.tile([C, CH], F32, tag="gen2")
                    nc.tensor.matmul(bb[:],
                                     cc["wbc"][:, (2 * k) * C:(2 * k + 1) * C],
                                     xrhs(k, j), start=True, stop=True)
                    du = chk.tile([C, CH], BF16, tag="du")
                    if k in (0, 2):
                        nc.vector.tensor_tensor(du[:],
                                                D[k][:, j * CH:(j + 1) * CH],
                                                xrhs(k, j), OP.mult)
                        # k0/k2 run during the sigmoid ACT phase: read the
                        # B broadcast straight from PSUM on DVE
                        nc.vector.tensor_tensor(bso[:, j * CH:(j + 1) * CH],
                                                du[:], bb[:], OP.mult)
                    else:
                        nc.gpsimd.tensor_tensor(du[:],
                                                D[k][:, j * CH:(j + 1) * CH],
                                                xrhs(k, j), OP.mult)
                        # k1/k3 run after the sigmoid phase: ACT has slack,
                        # stage B via ACT so the DVE multiply runs 2x bf16
                        bbs = chk.tile([C, CH], BF16, tag="bb")
                        nc.scalar.activation(bbs[:], bb[:], AF.Identity)
                        nc.vector.tensor_tensor(bso[:, j * CH:(j + 1) * CH],
                                                du[:], bbs[:], OP.mult)
                h = scanp.tile([C, L], BF16, tag="sc")
                HL = L // 2
                if k in (0, 1):
                    # split scan with state carry: half 1 starts after only
                    # half the bso chunks; y-multiplies of the first half
                    # don't wait for the second
                    nc.vector.tensor_tensor_scan(h[:, 0:HL], dA[:, 0:HL],
                                                 bso[:, 0:HL], 0.0,
                                                 OP.mult, OP.add)
                    nc.vector.tensor_tensor_scan(h[:, HL:L], dA[:, HL:L],
                                                 bso[:, HL:L],
                                                 h[:, HL - 1:HL],
                                                 OP.mult, OP.add)
                else:
                    # reversed scan: scan positions 0..HL-1 are cols L-1..HL
                    nc.vector.tensor_tensor_scan(h[:, HL:L][:, ::-1],
                                                 dA[:, HL:L][:, ::-1],
                                                 bso[:, HL:L][:, ::-1], 0.0,
                                                 OP.mult, OP.add)
                    nc.vector.tensor_tensor_scan(h[:, 0:HL][:, ::-1],
                                                 dA[:, 0:HL][:, ::-1],
                                                 bso[:, 0:HL][:, ::-1],
                                                 h[:, HL:HL + 1],
                                                 OP.mult, OP.add)
                # y_k = h * Cs_b  (+ accumulate into l-major / w-major accs)
                if k in (0, 1):
                    dst = hcp.tile([C, L], BF16, tag="hc", name=f"hc{k}")
                else:
                    dst = accp.tile([C, L], BF16, tag="acc", name=f"acc{k}")
                for j in range(NCHUNK):
                    cb = ps.tile([C, CH], F32, tag="gen2")
                    nc.tensor.matmul(cb[:],
                                     cc["wbc"][:, (2 * k + 1) * C:(2 * k + 2) * C],
                                     xrhs(k, j), start=True, stop=True)
                    # stage C broadcast through ACT (idle in this phase) so
                    # the DVE multiplies run in 2x bf16 mode
                    cbs = chk.tile([C, CH], BF16, tag="bb")
                    nc.scalar.activation(cbs[:], cb[:], AF.Identity)
                    if k in (0, 1):
                        nc.vector.tensor_tensor(dst[:, j * CH:(j + 1) * CH],
                                                h[:, j * CH:(j + 1) * CH],
                                                cbs[:], OP.mult)
                    else:
                        tmp = chk.tile([C, CH], BF16, tag="du")
                        nc.vector.tensor_tensor(tmp[:],
                                                h[:, j * CH:(j + 1) * CH],
                                                cbs[:], OP.mult)
                        nc.vector.tensor_tensor(dst[:, j * CH:(j + 1) * CH],
                                                accs[k - 2][:,
                                                            j * CH:(j + 1) * CH],
                                                tmp[:], OP.add)
                accs[k] = dst

            # =============== cross-merge + D*u + LN + out_proj =============
            preln = hcp.tile([C, L], BF16, tag="hc")
            accT = accs[3][:].rearrange("c (w h) -> c w h", w=W).transpose(
                [0, 2, 1])
            for j in range(NCHUNK):
                t2 = chk.tile([C, CH], BF16, tag="du")
                nc.gpsimd.tensor_tensor(t2[:],
                                        accs[2][:, j * CH:(j + 1) * CH],
                                        accT[:, j * 8:(j + 1) * 8, :], OP.add)
                nc.vector.scalar_tensor_tensor(
                    preln[:, j * CH:(j + 1) * CH],
                    v4[:, j * CH:(j + 1) * CH], V96("Dsum"), t2[:],
                    OP.mult, OP.add)
            x2 = bigp.tile([C, L], F32, tag="x2")
            ynf = scanp.tile([C, L], BF16, tag="sc", name="ynf")
            for j in range(NCHUNK):
                ln_norm_chunk(preln[:, j * CH:(j + 1) * CH], False,
                              ynf[:, j * CH:(j + 1) * CH], sq_gp=True)
            for j in range(NCHUNK):
                po = ps.tile([C, CH], F32, tag="gen")
                nc.tensor.matmul(po[:], cc["wop"][:],
                                 ynf[:, j * CH:(j + 1) * CH], start=True,
                                 stop=True)
                nc.vector.scalar_tensor_tensor(x2[:, j * CH:(j + 1) * CH]
                                               .bitcast(F32R), po[:],
                                               V96("opb"),
                                               x1[:, j * CH:(j + 1) * CH],
                                               OP.add, OP.add)

            # =============== MLP block ===============
            outsb = bigp.tile([C, L], F32, tag="xres")
            mlp_block(x2, True, "wmfc1", "wmfc2", "mfc1b", "mfc2b", x2, outsb,
                      sq_gp=False)
            nc.sync.dma_start(dout, outsb[:])

    nc.compile()
    return nc


def get_program_and_inputs(inputs):
    host, ix, a_is_neg1 = build_host_tensors(inputs)
    key = ("prog", a_is_neg1)
    if key not in _CACHE:
        _CACHE[key] = build_program(ix, a_is_neg1)
    nc = _CACHE[key]
    x = np.asarray(inputs["x"], np.float32)
    in_maps = []
    for b in range(B):
        m = {k: v for k, v in host.items()}
        m["xint"] = pad_image(x[b])
        # pre-round to the 2xbf16 (fp32r) representable set: this tile is
        # read by fp32r LN-stats matmuls before being overwritten in place
        xr = x[b].reshape(C, L).astype(np.float32)
        hi = xr.astype(bfnp).astype(np.float32)
        lo = (xr - hi).astype(bfnp).astype(np.float32)
        m["xres"] = hi + lo
        in_maps.append(m)
    return nc, in_maps


def kernel(**inputs):
    nc, in_maps = get_program_and_inputs(inputs)
    res = run_bass_kernel_spmd(nc, in_maps, list(range(B)))
    out = np.stack([res.results[b]["out"].reshape(C, H, W) for b in range(B)])
    return out.astype(np.float32)


if __name__ == "__main__":
    # smoke build
    host, ix, a_neg1 = build_host_tensors(
        {k: np.zeros(s, np.float32) for k, s in [  # noqa

            ("x", (B, C, H, W)), ("cn_dw_w", (C, 7, 7)), ("cn_dw_b", (C,)),
            ("cn_ln_w", (C,)), ("cn_ln_b", (C,)), ("cn_fc1_w", (4 * C, C)),
            ("cn_fc1_b", (4 * C,)), ("cn_fc2_w", (C, 4 * C)), ("cn_fc2_b", (C,)),
            ("v_ln1_w", (C,)), ("v_ln1_b", (C,)), ("ip_w", (C, C)),
            ("ip_b", (C,)), ("dw_w", (C, 3, 3)), ("dw_b", (C,)),
            ("x_proj_w", (K, R + 2 * N, C)), ("dt_w", (K, C, R)),
            ("dt_b", (K, C)), ("A_logs", (K * C, N)), ("Ds", (K * C,)),
            ("o_ln_w", (C,)), ("o_ln_b", (C,)), ("op_w", (C, C)),
            ("op_b", (C,)), ("v_ln2_w", (C,)), ("v_ln2_b", (C,)),
            ("m_fc1_w", (4 * C, C)), ("m_fc1_b", (4 * C,)),
            ("m_fc2_w", (C, 4 * C)), ("m_fc2_b", (C,)),
        ]})
    nc = build_program(ix, a_neg1)
    print("program built OK:", len(list(nc.all_instructions())), "instructions")
